# revision 12
# baseline (speedup 1.0000x reference)
"""KitNET anomaly-detection ensemble (25 tiny tied-weight autoencoders) on 8 Trainium2 cores.

v2 strategy (vs v1 block-dense):
  - Host prep does the gather + transpose + bf16 cast: x is reorganized into an
    AE-grouped, feature-major layout xt[chunk, feat_row, sample] so the device
    needs NO PE transposes and NO fp32->bf16 casts, and DMA traffic halves.
  - AEs are grouped 7+7+7+4 into 4 feature chunks (7 AEs x 16 feats = 112 rows).
    Encode / decode / group-sum are then BLOCK-DIAGONAL: one matmul per chunk
    (cost on PE is moving-dim columns only), so 4 passes each instead of 12.
  - Hidden is padded to 16 per AE (zero weight rows/cols kill the junk lanes).
  - Biases fold into the matmuls via a ones-row in xt (row 112 / 64) carrying
    hb, plus a "sigma(30)=1" generator column so ht gets its own ones-row for
    the decode-side vb. Works for any hb/vb at zero extra cost.
  - Sigmoids run as ONE ACT instruction per stage spanning all 4 PSUM banks.
  - err^2 on DVE in-place; per-AE sums via a G matmul (tile_position col
    strips) into a reused PSUM bank; sqrt is batched at the end (single ACT
    table switch) and the 25-AE reduction is a mask-matmul with rmse
    stationary, which lands y sample-major across 128 partitions.
  - Samples are shuffled on host (b = p*128 + sb*32 + t at position (t, sb*128+p))
    so the final y DMA writes contiguous 512B partition lines.
"""

import sys

for _p in ("/opt/trn_rl_repo", "/opt/pypackages"):
    if _p not in sys.path:
        sys.path.append(_p)

import numpy as np

B = 131072
F = 400          # features
N_AE = 25
KF = 16          # features per AE
H = 12           # hidden per AE
HP = 16          # hidden padded per AE
EPS = 1e-6
N_CORES = 8
BC = B // N_CORES    # 16384 samples per core
NB = 512             # batch tile (matmul moving free dim)
NT = BC // NB        # 32 tiles per core

_AE_CH = (7, 7, 7, 4)            # AEs per chunk
_K_CH = (113, 113, 113, 65)      # contraction rows per chunk (feats + ones row)
_NF_CH = (112, 112, 112, 64)     # real feature rows per chunk
M_ENC = 113                      # enc out cols: 112 padded hidden + ones-gen
BIG = 30.0                       # sigmoid(30) == 1.0 (ones generator)

_NC_CACHE = {}


def _build_nc():
    import concourse.tile as tile
    from concourse import bacc, mybir

    f32 = mybir.dt.float32
    bf16 = mybir.dt.bfloat16
    AF = mybir.ActivationFunctionType

    nc = bacc.Bacc()

    xt_d = nc.declare_dram_parameter("xt", [128, 4, BC], bf16, isOutput=False)
    wenc_d = nc.declare_dram_parameter("wenc", [113, 4, M_ENC], bf16, isOutput=False)
    wdec_d = nc.declare_dram_parameter("wdec", [113, 4, 112], bf16, isOutput=False)
    g_d = nc.declare_dram_parameter("gmat", [112, 4, 32], bf16, isOutput=False)
    mask_d = nc.declare_dram_parameter("mask", [128, 1], bf16, isOutput=False)
    y_d = nc.declare_dram_parameter("y", [BC], f32, isOutput=True)

    with tile.TileContext(nc) as tc:
        with (
            tc.tile_pool(name="singles", bufs=1) as singles,
            tc.tile_pool(name="xt", bufs=4) as xt_p,
            tc.tile_pool(name="ht", bufs=2) as ht_p,
            tc.tile_pool(name="rec", bufs=2) as rec_p,
            tc.tile_pool(name="encp", bufs=1, space="PSUM") as encp_p,
            tc.tile_pool(name="decp", bufs=1, space="PSUM") as decp_p,
        ):
            # --- constants / weights ---
            wenc = singles.tile([113, 4, M_ENC], bf16)
            nc.sync.dma_start(out=wenc, in_=wenc_d[:, :, :])
            wdec = singles.tile([113, 4, 112], bf16)
            nc.sync.dma_start(out=wdec, in_=wdec_d[:, :, :])
            gm = singles.tile([112, 4, 32], bf16)
            nc.sync.dma_start(out=gm, in_=g_d[:, :, :])
            mask = singles.tile([128, 1], bf16)
            nc.sync.dma_start(out=mask, in_=mask_d[:, :])
            eps_sb = singles.tile([128, 1], f32)
            nc.vector.memset(eps_sb, EPS)
            # per-tile per-AE sums of squared err, fp32 [strip, tile, sample]
            sall = singles.tile([128, NT, NB], f32)
            rmse = singles.tile([128, NT, NB], bf16)
            ysb = singles.tile([128, NT * 4], f32)

            # 3-stage software pipeline: slot s runs enc(s) || dec(s-1) ||
            # err/G/S(s-2) so ACT streams back-to-back and PE stays warm.
            xts = {}    # tile -> xt tile
            hts = {}    # tile -> ht tile
            recs = {}   # tile -> (rec tile, decp tile)

            for s in range(NT + 2):
                # ---- DMA prefetch for tile s
                if s < NT:
                    xt = xt_p.tile([128, 4, NB], bf16, tag="xt")
                    nc.sync.dma_start(
                        out=xt, in_=xt_d[:, :, s * NB : (s + 1) * NB]
                    )
                    xts[s] = xt

                # ---- stage C1: err elementwise of tile s-2 (Pool + DVE)
                tt = s - 2
                if tt >= 0:
                    pxt = xts.pop(tt)
                    prec, pdecp = recs.pop(tt)
                    # d = xt - rec; d *= d — both on DVE (GPSIMD measured ~3.5x
                    # slower for tensor_tensor and sat on the critical path)
                    nc.vector.tensor_sub(
                        prec[0:112, :, :], pxt[0:112, :, :], prec[0:112, :, :]
                    )
                    nc.vector.tensor_mul(
                        prec[0:112, :, :], prec[0:112, :, :], prec[0:112, :, :]
                    )

                # ---- stage A: encode tile s
                if s < NT:
                    encp = encp_p.tile([128, 4, NB], f32, tag="encp")
                    for c in range(4):
                        k = _K_CH[c]
                        nc.tensor.matmul(
                            encp[0:M_ENC, c, :],
                            lhsT=wenc[0:k, c, :],
                            rhs=xts[s][0:k, c, :],
                            start=True,
                            stop=True,
                        )
                    ht = ht_p.tile([128, 4, NB], bf16, tag="ht")
                    nc.scalar.activation(
                        out=ht[0:M_ENC, :, :],
                        in_=encp[0:M_ENC, :, :],
                        func=AF.Sigmoid,
                    )
                    hts[s] = ht

                # ---- stage B: decode tile s-1
                tt = s - 1
                if 0 <= tt < NT:
                    pht = hts.pop(tt)
                    cur_decp = decp_p.tile([128, 4, NB], f32, tag="decp")
                    for c in range(4):
                        k = _K_CH[c]
                        nc.tensor.matmul(
                            cur_decp[0:112, c, :],
                            lhsT=wdec[0:k, c, :],
                            rhs=pht[0:k, c, :],
                            start=True,
                            stop=True,
                        )
                    rec = rec_p.tile([128, 4, NB], bf16, tag="rec")
                    nc.scalar.activation(
                        out=rec[0:112, :, :],
                        in_=cur_decp[0:112, :, :],
                        func=AF.Sigmoid,
                    )
                    recs[tt] = (rec, cur_decp)

                # ---- stage C2: G-matmuls + S copy of tile s-2, into bank 3
                # of decp(s-1) AFTER its dec-ACT read — keeps the err path off
                # the encode critical chain (enc only waits its own ACT) and
                # psum at exactly 8 banks.
                tt = s - 2
                if tt >= 0:
                    for c in range(4):
                        kg = _NF_CH[c]
                        nc.tensor.matmul(
                            cur_decp[32 * c : 32 * (c + 1), 3, :],
                            lhsT=gm[0:kg, c, :],
                            rhs=prec[0:kg, c, :],
                            start=True,
                            stop=True,
                            tile_position=(0, 32 * c),
                        )
                    nc.vector.tensor_copy(out=sall[:, tt, :], in_=cur_decp[:, 3, :])

            # ---- phase B: rmse = sqrt(S/16 + eps) batched (one table switch),
            #      then y = mask^T-weighted partition sum with rmse stationary
            yp = encp_p.tile([128, 4, NB], f32, tag="encp")
            # Guard: eps bias recomputed from the LAST tile's S column so every
            # sqrt (and hence the whole phase) is ordered after the main loop —
            # otherwise the scheduler hoists sqrts into the loop and thrashes
            # the ACT table (sigmoid<->sqrt reload is 1283 ns each way).
            eps_g = singles.tile([128, 1], f32)
            nc.vector.tensor_scalar(
                eps_g,
                sall[:, NT - 1, 0:1],
                0.0,
                EPS,
                mybir.AluOpType.mult,
                mybir.AluOpType.add,
            )
            GRP = 2
            for g in range(NT // GRP):
                nc.scalar.activation(
                    out=rmse[:, g * GRP : (g + 1) * GRP, :],
                    in_=sall[:, g * GRP : (g + 1) * GRP, :],
                    func=AF.Sqrt,
                    bias=eps_g,
                    scale=1.0 / KF,
                )
                for tt in range(g * GRP, (g + 1) * GRP):
                    for sb in range(4):
                        f = sb * 32 + tt
                        nc.tensor.matmul(
                            yp[0:128, 0, f : f + 1],
                            lhsT=rmse[0:128, tt, sb * 128 : (sb + 1) * 128],
                            rhs=mask[0:128, 0:1],
                            start=True,
                            stop=True,
                        )
            nc.vector.tensor_copy(out=ysb, in_=yp[:, 0, 0 : NT * 4])
            # y[b], b = p*128 + sb*32 + t  ->  partition-contiguous lines
            nc.sync.dma_start(
                out=y_d[:].rearrange("(p f) -> p f", p=128), in_=ysb
            )

    nc.compile()
    return nc


def _host_mats(W, hb, vb, idx):
    import ml_dtypes

    bf16 = ml_dtypes.bfloat16
    W = np.asarray(W, np.float32)
    hb = np.asarray(hb, np.float32)
    vb = np.asarray(vb, np.float32)
    idx = np.asarray(idx)

    # chunk c holds AEs ae0..ae0+n (7,7,7,4); feature row a_loc*16+k,
    # hidden (psum col / ht row) a_loc*16+h; ones-row at row 112 (c3: 64)
    wenc = np.zeros((113, 4, M_ENC), np.float32)
    wdec = np.zeros((113, 4, 112), np.float32)
    gmat = np.zeros((112, 4, 32), np.float32)
    mask = np.zeros((128, 1), np.float32)
    ae0 = 0
    for c, nae in enumerate(_AE_CH):
        ones_r = _NF_CH[c]  # 112 or 64
        for j in range(nae):
            a = ae0 + j
            for k in range(KF):
                r = j * KF + k
                wenc[r, c, j * HP : j * HP + H] = W[a, k, :]
                wdec[j * HP : j * HP + H, c, j * KF + k] = W[a, k, :]
                gmat[r, c, j] = 1.0
                wdec[ones_r, c, j * KF + k] = vb[a, k]
            wenc[ones_r, c, j * HP : j * HP + H] = hb[a, :]
            mask[32 * c + j, 0] = 1.0
        # ones generator: psum col (112 / 64) = 30 -> sigmoid = 1.0 in ht
        wenc[ones_r, c, ones_r] = BIG
        ae0 += nae

    return {
        "wenc": np.ascontiguousarray(wenc.astype(bf16)),
        "wdec": np.ascontiguousarray(wdec.astype(bf16)),
        "gmat": np.ascontiguousarray(gmat.astype(bf16)),
        "mask": np.ascontiguousarray(mask.astype(bf16)),
    }


def _host_x(x, idx):
    """Gather + transpose + shuffle + cast: returns per-core xt [4, 113, BC]."""
    import ml_dtypes

    bf16 = ml_dtypes.bfloat16
    perm = np.asarray(idx).reshape(-1)  # grouped feature order (25*16)
    # sample position (t, sb*128 + p) holds original sample p*128 + sb*32 + t
    p_ = np.arange(128)
    sb_ = np.arange(4)
    t_ = np.arange(NT)
    # bidx[t, sb*128+p] = p*128 + sb*32 + t
    bidx = (
        p_[None, None, :] * 128 + sb_[None, :, None] * 32 + t_[:, None, None]
    ).reshape(NT, 512).reshape(-1)  # [BC] position -> original sample

    outs = []
    for c in range(N_CORES):
        xc = x[c * BC : (c + 1) * BC]          # [BC, F] fp32
        xs = xc[bidx][:, perm]                  # shuffled samples, grouped feats
        xtc = np.zeros((128, 4, BC), np.float32)
        f0 = 0
        for ch, nae in enumerate(_AE_CH):
            nf = nae * KF
            xtc[0:nf, ch, :] = xs[:, f0 : f0 + nf].T
            xtc[_NF_CH[ch], ch, :] = 1.0        # ones-row (bias contraction)
            f0 += nf
        outs.append(np.ascontiguousarray(xtc.astype(bf16)))
    return outs


def _get_nc():
    if "nc" not in _NC_CACHE:
        _NC_CACHE["nc"] = _build_nc()
    return _NC_CACHE["nc"]


def _run(x, W, hb, vb, idx, trace=False):
    from concourse.bass_utils import run_bass_kernel_spmd

    x = np.asarray(x, np.float32)
    consts = _host_mats(W, hb, vb, idx)
    xts = _host_x(x, idx)
    in_maps = [{"xt": xts[c], **consts} for c in range(N_CORES)]
    nc = _get_nc()
    res = run_bass_kernel_spmd(nc, in_maps, list(range(N_CORES)), trace=trace)
    y = np.concatenate([res.results[c]["y"] for c in range(N_CORES)])
    return y, res


def kernel(x, W, hb, vb, idx):
    y, _ = _run(x, W, hb, vb, idx)
    return y


# revision 17
# speedup vs baseline: 1.0385x; 1.0385x over previous
"""KitNET anomaly-detection ensemble (25 tiny tied-weight autoencoders) on 8 Trainium2 cores.

v2 strategy (vs v1 block-dense):
  - Host prep does the gather + transpose + bf16 cast: x is reorganized into an
    AE-grouped, feature-major layout xt[chunk, feat_row, sample] so the device
    needs NO PE transposes and NO fp32->bf16 casts, and DMA traffic halves.
  - AEs are grouped 7+7+7+4 into 4 feature chunks (7 AEs x 16 feats = 112 rows).
    Encode / decode / group-sum are then BLOCK-DIAGONAL: one matmul per chunk
    (cost on PE is moving-dim columns only), so 4 passes each instead of 12.
  - Hidden is padded to 16 per AE (zero weight rows/cols kill the junk lanes).
  - Biases fold into the matmuls via a ones-row in xt (row 112 / 64) carrying
    hb, plus a "sigma(30)=1" generator column so ht gets its own ones-row for
    the decode-side vb. Works for any hb/vb at zero extra cost.
  - Sigmoids run as ONE ACT instruction per stage spanning all 4 PSUM banks.
  - err^2 on DVE in-place; per-AE sums via a G matmul (tile_position col
    strips) into a reused PSUM bank; sqrt is batched at the end (single ACT
    table switch) and the 25-AE reduction is a mask-matmul with rmse
    stationary, which lands y sample-major across 128 partitions.
  - Samples are shuffled on host (b = p*128 + sb*32 + t at position (t, sb*128+p))
    so the final y DMA writes contiguous 512B partition lines.
"""

import sys

for _p in ("/opt/trn_rl_repo", "/opt/pypackages"):
    if _p not in sys.path:
        sys.path.append(_p)

import numpy as np

B = 131072
F = 400          # features
N_AE = 25
KF = 16          # features per AE
H = 12           # hidden per AE
HP = 16          # hidden padded per AE
EPS = 1e-6
N_CORES = 8
BC = B // N_CORES    # 16384 samples per core
NB = 512             # batch tile (matmul moving free dim)
NT = BC // NB        # 32 tiles per core

_AE_CH = (7, 7, 7, 4)            # AEs per chunk
_K_CH = (113, 113, 113, 65)      # contraction rows per chunk (feats + ones row)
_NF_CH = (112, 112, 112, 64)     # real feature rows per chunk
M_ENC = 113                      # enc out cols: 112 padded hidden + ones-gen
BIG = 30.0                       # sigmoid(30) == 1.0 (ones generator)

_NC_CACHE = {}


def _build_nc():
    import concourse.tile as tile
    from concourse import bacc, mybir

    f32 = mybir.dt.float32
    bf16 = mybir.dt.bfloat16
    AF = mybir.ActivationFunctionType

    nc = bacc.Bacc()

    xt_d = nc.declare_dram_parameter("xt", [128, 4, BC], bf16, isOutput=False)
    # all weights in one 128-partition blob => one sprayed DMA at startup
    # cols: [0:452) wenc (4x113), [452:900) wdec (4x112), [900:1028) gm (4x32),
    #       [1028] mask
    wall_d = nc.declare_dram_parameter("wall", [128, 1029], bf16, isOutput=False)
    y_d = nc.declare_dram_parameter("y", [BC], f32, isOutput=True)

    with tile.TileContext(nc) as tc:
        with (
            tc.tile_pool(name="singles", bufs=1) as singles,
            tc.tile_pool(name="xt", bufs=5) as xt_p,
            tc.tile_pool(name="ht", bufs=2) as ht_p,
            tc.tile_pool(name="rec", bufs=2) as rec_p,
            tc.tile_pool(name="encp", bufs=1, space="PSUM") as encp_p,
            tc.tile_pool(name="decp", bufs=1, space="PSUM") as decp_p,
        ):
            # --- constants / weights (one blob DMA) ---
            wall = singles.tile([128, 1029], bf16)
            nc.sync.dma_start(out=wall, in_=wall_d[:, :])

            def wenc_ap(c):
                return wall[0 : _K_CH[c], c * M_ENC : (c + 1) * M_ENC]

            def wdec_ap(c):
                return wall[0 : _K_CH[c], 452 + c * 112 : 452 + (c + 1) * 112]

            def gm_ap(c):
                return wall[0 : _NF_CH[c], 900 + c * 32 : 900 + (c + 1) * 32]

            mask = wall[0:128, 1028:1029]
            # per-tile per-AE sums of squared err, fp32 [strip, tile, sample]
            sall = singles.tile([128, NT, NB], f32)
            rmse = singles.tile([128, NT, NB], bf16)
            ysb = singles.tile([128, NT * 4], f32)

            # 4-stage software pipeline: slot s runs
            #   C1: err elementwise of tile s-3 (DVE)
            #   B2: dec-sigmoid of tile s-2 (ACT; its matmuls ran last slot)
            #   A : enc matmuls + sigmoid of tile s
            #   C2: G-matmuls + S copy of tile s-3 (into decp bank 3 after B2)
            #   B1: dec matmuls of tile s-1
            # so both ACT instructions of a slot have slot-old inputs and the
            # ACT engine streams back-to-back at its ~3.9us/tile floor.
            xts = {}    # tile -> xt tile
            hts = {}    # tile -> ht tile
            decps = {}  # tile -> decp psum tile (matmuls done, ACT pending)
            recs = {}   # tile -> rec tile
            g_target = None

            for s in range(NT + 3):
                # ---- DMA prefetch for tile s
                if s < NT:
                    xt = xt_p.tile([128, 4, NB], bf16, tag="xt")
                    nc.sync.dma_start(
                        out=xt, in_=xt_d[:, :, s * NB : (s + 1) * NB]
                    )
                    xts[s] = xt

                # ---- stage C1: err elementwise of tile s-3 (DVE)
                te = s - 3
                if te >= 0:
                    pxt = xts.pop(te)
                    prec = recs.pop(te)
                    nc.vector.tensor_sub(
                        prec[0:112, :, :], pxt[0:112, :, :], prec[0:112, :, :]
                    )
                    nc.vector.tensor_mul(
                        prec[0:112, :, :], prec[0:112, :, :], prec[0:112, :, :]
                    )

                # ---- stage B2: dec sigmoid of tile s-2
                td = s - 2
                if 0 <= td < NT:
                    g_target = decps.pop(td)
                    rec = rec_p.tile([128, 4, NB], bf16, tag="rec")
                    nc.scalar.activation(
                        out=rec[0:112, :, :],
                        in_=g_target[0:112, :, :],
                        func=AF.Sigmoid,
                    )
                    recs[td] = rec

                # ---- stage A: encode tile s
                if s < NT:
                    encp = encp_p.tile([128, 4, NB], f32, tag="encp")
                    for c in range(4):
                        k = _K_CH[c]
                        nc.tensor.matmul(
                            encp[0:M_ENC, c, :],
                            lhsT=wenc_ap(c),
                            rhs=xts[s][0:k, c, :],
                            start=True,
                            stop=True,
                        )
                    ht = ht_p.tile([128, 4, NB], bf16, tag="ht")
                    nc.scalar.activation(
                        out=ht[0:M_ENC, :, :],
                        in_=encp[0:M_ENC, :, :],
                        func=AF.Sigmoid,
                    )
                    hts[s] = ht

                # ---- stage C2: G-matmuls + S copy of tile s-3 into bank 3 of
                # the psum tile stage B2 just read (G writes after that read)
                if te >= 0:
                    for c in range(4):
                        kg = _NF_CH[c]
                        nc.tensor.matmul(
                            g_target[32 * c : 32 * (c + 1), 3, :],
                            lhsT=gm_ap(c),
                            rhs=prec[0:kg, c, :],
                            start=True,
                            stop=True,
                            tile_position=(0, 32 * c),
                        )
                    nc.vector.tensor_copy(out=sall[:, te, :], in_=g_target[:, 3, :])

                # ---- stage B1: decode matmuls of tile s-1
                tm = s - 1
                if 0 <= tm < NT:
                    pht = hts.pop(tm)
                    decp = decp_p.tile([128, 4, NB], f32, tag="decp")
                    for c in range(4):
                        k = _K_CH[c]
                        nc.tensor.matmul(
                            decp[0:112, c, :],
                            lhsT=wdec_ap(c),
                            rhs=pht[0:k, c, :],
                            start=True,
                            stop=True,
                        )
                    decps[tm] = decp

            # ---- phase B: rmse = sqrt(S/16 + eps) batched (one table switch),
            #      then y = mask^T-weighted partition sum with rmse stationary
            yp = encp_p.tile([128, 4, NB], f32, tag="encp")
            # Guard: eps bias recomputed from the LAST tile's S column so every
            # sqrt (and hence the whole phase) is ordered after the main loop —
            # otherwise the scheduler hoists sqrts into the loop and thrashes
            # the ACT table (sigmoid<->sqrt reload is 1283 ns each way).
            eps_g = singles.tile([128, 1], f32)
            nc.vector.tensor_scalar(
                eps_g,
                sall[:, NT - 1, 0:1],
                0.0,
                EPS,
                mybir.AluOpType.mult,
                mybir.AluOpType.add,
            )
            GRP = 2
            for g in range(NT // GRP):
                nc.scalar.activation(
                    out=rmse[:, g * GRP : (g + 1) * GRP, :],
                    in_=sall[:, g * GRP : (g + 1) * GRP, :],
                    func=AF.Sqrt,
                    bias=eps_g,
                    scale=1.0 / KF,
                )
                for tt in range(g * GRP, (g + 1) * GRP):
                    for sb in range(4):
                        f = sb * 32 + tt
                        nc.tensor.matmul(
                            yp[0:128, 0, f : f + 1],
                            lhsT=rmse[0:128, tt, sb * 128 : (sb + 1) * 128],
                            rhs=mask[0:128, 0:1],
                            start=True,
                            stop=True,
                        )
            nc.vector.tensor_copy(out=ysb, in_=yp[:, 0, 0 : NT * 4])
            # y[b], b = p*128 + sb*32 + t  ->  partition-contiguous lines
            nc.sync.dma_start(
                out=y_d[:].rearrange("(p f) -> p f", p=128), in_=ysb
            )

    nc.compile()
    return nc


def _host_mats(W, hb, vb, idx):
    import ml_dtypes

    bf16 = ml_dtypes.bfloat16
    W = np.asarray(W, np.float32)
    hb = np.asarray(hb, np.float32)
    vb = np.asarray(vb, np.float32)
    idx = np.asarray(idx)

    # chunk c holds AEs ae0..ae0+n (7,7,7,4); feature row a_loc*16+k,
    # hidden (psum col / ht row) a_loc*16+h; ones-row at row 112 (c3: 64)
    wenc = np.zeros((113, 4, M_ENC), np.float32)
    wdec = np.zeros((113, 4, 112), np.float32)
    gmat = np.zeros((112, 4, 32), np.float32)
    mask = np.zeros((128, 1), np.float32)
    ae0 = 0
    for c, nae in enumerate(_AE_CH):
        ones_r = _NF_CH[c]  # 112 or 64
        for j in range(nae):
            a = ae0 + j
            for k in range(KF):
                r = j * KF + k
                wenc[r, c, j * HP : j * HP + H] = W[a, k, :]
                wdec[j * HP : j * HP + H, c, j * KF + k] = W[a, k, :]
                gmat[r, c, j] = 1.0
                wdec[ones_r, c, j * KF + k] = vb[a, k]
            wenc[ones_r, c, j * HP : j * HP + H] = hb[a, :]
            mask[32 * c + j, 0] = 1.0
        # ones generator: psum col (112 / 64) = 30 -> sigmoid = 1.0 in ht
        wenc[ones_r, c, ones_r] = BIG
        ae0 += nae

    wall = np.zeros((128, 1029), np.float32)
    for c in range(4):
        wall[0:113, c * M_ENC : (c + 1) * M_ENC] = wenc[:, c, :]
        wall[0:113, 452 + c * 112 : 452 + (c + 1) * 112] = wdec[:, c, :]
        wall[0:112, 900 + c * 32 : 900 + (c + 1) * 32] = gmat[:, c, :]
    wall[:, 1028] = mask[:, 0]
    return {"wall": np.ascontiguousarray(wall.astype(bf16))}


def _host_x(x, idx):
    """Gather + transpose + shuffle + cast: returns per-core xt [4, 113, BC]."""
    import ml_dtypes

    bf16 = ml_dtypes.bfloat16
    perm = np.asarray(idx).reshape(-1)  # grouped feature order (25*16)
    # sample position (t, sb*128 + p) holds original sample p*128 + sb*32 + t
    p_ = np.arange(128)
    sb_ = np.arange(4)
    t_ = np.arange(NT)
    # bidx[t, sb*128+p] = p*128 + sb*32 + t
    bidx = (
        p_[None, None, :] * 128 + sb_[None, :, None] * 32 + t_[:, None, None]
    ).reshape(NT, 512).reshape(-1)  # [BC] position -> original sample

    outs = []
    for c in range(N_CORES):
        xc = x[c * BC : (c + 1) * BC]          # [BC, F] fp32
        xs = xc[bidx][:, perm]                  # shuffled samples, grouped feats
        xtc = np.zeros((128, 4, BC), np.float32)
        f0 = 0
        for ch, nae in enumerate(_AE_CH):
            nf = nae * KF
            xtc[0:nf, ch, :] = xs[:, f0 : f0 + nf].T
            xtc[_NF_CH[ch], ch, :] = 1.0        # ones-row (bias contraction)
            f0 += nf
        outs.append(np.ascontiguousarray(xtc.astype(bf16)))
    return outs


def _get_nc():
    if "nc" not in _NC_CACHE:
        _NC_CACHE["nc"] = _build_nc()
    return _NC_CACHE["nc"]


def _run(x, W, hb, vb, idx, trace=False):
    from concourse.bass_utils import run_bass_kernel_spmd

    x = np.asarray(x, np.float32)
    consts = _host_mats(W, hb, vb, idx)
    xts = _host_x(x, idx)
    in_maps = [{"xt": xts[c], **consts} for c in range(N_CORES)]
    nc = _get_nc()
    res = run_bass_kernel_spmd(nc, in_maps, list(range(N_CORES)), trace=trace)
    y = np.concatenate([res.results[c]["y"] for c in range(N_CORES)])
    return y, res


def kernel(x, W, hb, vb, idx):
    y, _ = _run(x, W, hb, vb, idx)
    return y
